# revision 3
# baseline (speedup 1.0000x reference)
"""Trainium2 Bass kernel for AnatomicalMaskedLinear (block-masked dense layer).

Reference op:
    mask  = kron(adjacency, ones(256, 128))            # (21*256, 21*128)
    y     = x.reshape(B, 21*128) @ (weight*mask).T + bias
    out   = y.reshape(B, 21, 256)

Strategy (v2):
  * Only nonzero (256o x 128k) blocks are shipped/matmul'd (S=nnz blocks).
  * 8 cores = 4 batch quarters x 2 node-row halves; all cores share one
    SPMD graph (same adjacency-derived schedule), only data differs.
  * bf16 operands, 1024-wide moving operand: one matmul per (node, j)
    block into a 2-bank PSUM tile [128, 1024] -> 237 matmuls/core instead
    of 474 (less NX issue overhead, fewer LDWEIGHTS).
  * PE warm-up: HAM clock gate holds the PE at 1.2 GHz until ~3.4us of
    sustained activity. A memset + dummy-matmul burst issued before the
    first real data arrives flips the gate early so the real stream runs
    at 2.4 GHz almost immediately.
  * All loads ride the sync HWDGE ring in first-use order (x is packed
    host-side in first-use block order so bulk triggers are contiguous);
    the first node's blocks use per-block triggers so the first matmul
    fires as soon as ~0.5 MB has landed instead of waiting for a bulk
    transfer. Everything stays resident in SBUF (13 MB total).
  * PSUM evacuation (bias add + f32->bf16 cast, activation Identity with
    per-partition bias) AND the store trigger both run on the scalar
    engine: evac->store needs no cross-engine event, and output traffic
    halves (bf16). Host upcasts to f32.
  * Fewer DMA triggers + fewer cross-engine events shrink the end-of-NEFF
    event-semaphore drain ladder, which counts toward measured exec time.
"""

import os
import numpy as np
import ml_dtypes

NUM_NODES = 21
IN_F = 128
OUT_F = 256
BATCH = 4096
N_CORES = 8
P_BATCH = 4                      # batch ways
B_C = BATCH // P_BATCH           # 1024 batch rows per core
K_TOTAL = NUM_NODES * IN_F       # 2688
O_C = NUM_NODES * 128            # 2688 out rows per core (half of each node)
N_WARM = 16                      # dummy matmuls to open the HAM clock gate

_CACHE = {}                      # schedule key -> (nc, sched, jorder)


def _node_order(active):
    """Greedy: minimize newly-required x blocks at each step."""
    loaded = set()
    remaining = set(range(NUM_NODES))
    order = []
    while remaining:
        nxt = min(remaining,
                  key=lambda i: (len(set(active[i]) - loaded), len(active[i]), i))
        order.append(nxt)
        loaded |= set(active[nxt])
        remaining.remove(nxt)
    return order


def _build_schedule(adjacency):
    """[(i, [j...], zero_pad)] in greedy node order; >=1 slot per node."""
    A = np.asarray(adjacency) != 0
    active = {i: [int(j) for j in np.where(A[i])[0]] for i in range(NUM_NODES)}
    sched = []
    for i in _node_order(active):
        js = active[i]
        if js:
            sched.append((i, tuple(js), False))
        else:
            sched.append((i, None, True))
    # first-use order of x blocks; zero-pad slots use jorder[0]
    jorder = []
    seen = set()
    for i, js, zero in sched:
        if zero:
            continue
        for j in js:
            if j not in seen:
                seen.add(j)
                jorder.append(j)
    if not jorder:
        jorder = [0]
    sched = tuple((i, js if js is not None else (jorder[0],), zero)
                  for i, js, zero in sched)
    return sched, tuple(jorder)


def _w_chunks(sched):
    """Group schedule positions into w DMA chunks: per-node for the first
    few nodes, then ~1MB merged chunks."""
    sizes = [len(js) for _, js, _ in sched]
    chunks = []  # (start_slot, n_slots, last_sched_idx)
    k = 0
    s0 = 0
    while k < len(sched):
        if k < 7:
            n = sizes[k]
            last = k
            k += 1
        else:
            n = 0
            last = k
            while k < len(sched) and (n == 0 or n + sizes[k] <= 34):
                n += sizes[k]
                last = k
                k += 1
        chunks.append((s0, n, last))
        s0 += n
    return chunks


def _build_graph(sched, jorder):
    import concourse.tile as tile
    from concourse import bacc, mybir

    S = sum(len(js) for _, js, _ in sched)
    f32 = mybir.dt.float32
    bf16 = mybir.dt.bfloat16
    pos = {j: p for p, j in enumerate(jorder)}
    NJ = len(jorder)

    nc = bacc.Bacc("TRN2", target_bir_lowering=False, debug=False,
                   num_devices=N_CORES)

    xt_d = nc.declare_dram_parameter("xt", [128, NJ * B_C], bf16, isOutput=False)
    wp_d = nc.declare_dram_parameter("wp", [128, S * 128], bf16, isOutput=False)
    bias_d = nc.declare_dram_parameter("biasr", [128, NUM_NODES], f32,
                                       isOutput=False)
    out_d = nc.declare_dram_parameter("out", [O_C, B_C], bf16, isOutput=True)

    # per-node newly needed x blocks (contiguous in jorder by construction)
    new_js = []
    seen = set()
    for i, js, zero in sched:
        cur = [] if zero else [j for j in js if j not in seen]
        seen |= set(cur)
        new_js.append(cur)

    wchunks = _w_chunks(sched)

    with tile.TileContext(nc) as tc:
        with (
            tc.tile_pool(name="const", bufs=1) as constp,
            tc.tile_pool(name="persist", bufs=1) as persist,
            tc.tile_pool(name="psum", bufs=3, space="PSUM") as psump,
            tc.tile_pool(name="wps", bufs=1, space="PSUM") as warmpp,
            tc.tile_pool(name="outp", bufs=8) as outp,
        ):
            # --- PE warm-up: memset a small bf16 tile, then dummy matmuls ---
            warm_sb = constp.tile([128, 128], bf16)
            nc.gpsimd.memset(warm_sb[:], 0.0)
            warm_ps = warmpp.tile([128, 128], f32)
            for t in range(N_WARM):
                nc.tensor.matmul(warm_ps[:], warm_sb[:], warm_sb[:],
                                 start=True, stop=True)

            bias_sb = constp.tile([128, NUM_NODES], f32)
            xt_bf = persist.tile([128, NJ * B_C], bf16)
            wp_sb = persist.tile([128, S * 128], bf16)

            # --- load plan: all on the sync HWDGE ring, first-use order ---
            nc.sync.dma_start(out=bias_sb[:], in_=bias_d[:])

            emitted_w = 0   # how many w chunks issued
            def emit_w_chunk():
                nonlocal emitted_w
                if emitted_w < len(wchunks):
                    s0, n, _last = wchunks[emitted_w]
                    nc.sync.dma_start(out=wp_sb[:, s0 * 128:(s0 + n) * 128],
                                      in_=wp_d[:, s0 * 128:(s0 + n) * 128])
                    emitted_w += 1

            def emit_x(node_idx, per_block):
                cur = new_js[node_idx]
                if not cur:
                    return
                p0 = pos[cur[0]]
                n = len(cur)
                if per_block:
                    for t in range(n):
                        nc.sync.dma_start(
                            out=xt_bf[:, (p0 + t) * B_C:(p0 + t + 1) * B_C],
                            in_=xt_d[:, (p0 + t) * B_C:(p0 + t + 1) * B_C])
                else:
                    nc.sync.dma_start(
                        out=xt_bf[:, p0 * B_C:(p0 + n) * B_C],
                        in_=xt_d[:, p0 * B_C:(p0 + n) * B_C])

            # interleave: w chunk for node k, then its new x blocks
            emit_w_chunk()                      # node 0's w
            emit_x(0, per_block=True)
            k = 1
            while emitted_w < len(wchunks):
                _s0, _n, last = wchunks[emitted_w]
                emit_w_chunk()
                while k <= last:
                    emit_x(k, per_block=False)
                    k += 1

            # --- compute: one chain of 1024-wide matmuls per node ---
            s0 = 0
            n_sched = len(sched)
            for kk, (i, js, _zero) in enumerate(sched):
                nj = len(js)
                ps = psump.tile([128, B_C], f32, tag="acc", name=f"acc_{i}")
                for idx, j in enumerate(js):
                    w_ap = wp_sb[:, (s0 + idx) * 128:(s0 + idx + 1) * 128]
                    for bt in range(2):
                        nc.tensor.matmul(
                            ps[:, bt * 512:(bt + 1) * 512],
                            w_ap,
                            xt_bf[:, pos[j] * B_C + bt * 512:
                                  pos[j] * B_C + (bt + 1) * 512],
                            start=(idx == 0),
                            stop=(idx == nj - 1),
                        )
                ot = outp.tile([128, B_C], bf16, tag="ot")
                if kk >= n_sched - 1:
                    # split the last evac/store to shorten the serial tail
                    H = B_C // 2
                    for h in range(2):
                        nc.scalar.add(ot[:, h * H:(h + 1) * H],
                                      ps[:, h * H:(h + 1) * H],
                                      bias_sb[:, i:i + 1])
                        nc.scalar.dma_start(
                            out=out_d[i * 128:(i + 1) * 128, h * H:(h + 1) * H],
                            in_=ot[:, h * H:(h + 1) * H])
                else:
                    nc.scalar.add(ot[:], ps[:], bias_sb[:, i:i + 1])
                    nc.scalar.dma_start(
                        out=out_d[i * 128:(i + 1) * 128, :], in_=ot[:])
                s0 += nj

    nc.compile()
    return nc


def _get_graph(adjacency):
    sched, jorder = _build_schedule(adjacency)
    key = (sched, jorder)
    if key not in _CACHE:
        _CACHE[key] = (_build_graph(sched, jorder), sched, jorder)
    return _CACHE[key]


def _pack_inputs(x, weight, bias, sched, jorder):
    """Build the 8 per-core input maps (host-side slicing/layout only)."""
    bf16 = ml_dtypes.bfloat16
    x = np.asarray(x, dtype=np.float32)
    weight = np.asarray(weight, dtype=np.float32)
    bias = np.asarray(bias, dtype=np.float32)

    flat = []  # (i, j, zero) in slot order
    for i, js, zero in sched:
        for j in js:
            flat.append((i, j, zero))
    S = len(flat)

    w5 = weight.reshape(NUM_NODES, 2, 128, NUM_NODES, IN_F)  # i, h, o, j, k
    w5t = w5.transpose(1, 4, 0, 3, 2)                        # h, k, i, j, o

    si = np.array([f[0] for f in flat])
    sj = np.array([f[1] for f in flat])
    szero = np.array([f[2] for f in flat])

    wp_h = []
    for h in range(2):
        wp = np.ascontiguousarray(w5t[h][:, si, sj, :])      # [128, S, 128]
        if szero.any():
            wp[:, szero, :] = 0.0
        wp_h.append(wp.reshape(128, S * 128).astype(bf16))

    bias3 = bias.reshape(NUM_NODES, 2, 128)
    bias_h = [np.ascontiguousarray(bias3[:, h, :].T) for h in range(2)]

    jord = list(jorder)
    in_maps = []
    xt_cache = {}
    for c in range(N_CORES):
        bq, h = divmod(c, 2)
        if bq not in xt_cache:
            # [128 k, NJ blocks (first-use order), 1024 batch]
            xq = x[bq * B_C:(bq + 1) * B_C][:, jord, :]      # [1024, NJ, 128]
            xt_cache[bq] = np.ascontiguousarray(
                xq.transpose(2, 1, 0).reshape(128, len(jord) * B_C)).astype(bf16)
        in_maps.append({
            "xt": xt_cache[bq],
            "wp": wp_h[h],
            "biasr": bias_h[h],
        })
    return in_maps


def _gather_output(results):
    y = np.empty((P_BATCH, B_C, NUM_NODES, 2, 128), dtype=np.float32)
    for c in range(N_CORES):
        bq, h = divmod(c, 2)
        oc = results[c]["out"].astype(np.float32).reshape(NUM_NODES, 128, B_C)
        y[bq, :, :, h, :] = oc.transpose(2, 0, 1)
    return y.reshape(BATCH, NUM_NODES, OUT_F)


def _ensure_axon_profile_hook():
    """Provide antenv.axon_hooks if the image lacks it (no-op otherwise).

    concourse.bass_utils imports antenv.axon_hooks on the trace path; some
    images miss the module, which would turn BASS_TRACE=1 into an
    ImportError. Registers the standard ctypes NTFF hook when possible.
    """
    try:
        import antenv.axon_hooks  # noqa: F401
        return
    except ImportError:
        pass
    try:
        import antenv
    except ImportError:
        return
    import contextlib
    import ctypes
    import sys
    import types

    hook = None
    try:
        lib = ctypes.CDLL("/opt/axon/libaxon_pjrt.so")
        if hasattr(lib, "axon_start_nrt_profile"):
            lib.axon_start_nrt_profile.argtypes = [
                ctypes.POINTER(ctypes.c_int64), ctypes.c_size_t]
            lib.axon_start_nrt_profile.restype = ctypes.c_int64
            lib.axon_stop_nrt_profile.argtypes = [ctypes.c_char_p]
            lib.axon_stop_nrt_profile.restype = ctypes.c_int64

            @contextlib.contextmanager
            def hook(output_dir, device_ids):
                import jax
                jax.devices()
                if device_ids:
                    ids = (ctypes.c_int64 * len(device_ids))(*device_ids)
                    rc = lib.axon_start_nrt_profile(ids, len(device_ids))
                else:
                    rc = lib.axon_start_nrt_profile(None, 0)
                if rc != 0:
                    raise RuntimeError(f"axon_start_nrt_profile rc={rc}")
                try:
                    yield
                finally:
                    lib.axon_stop_nrt_profile(str(output_dir).encode())
    except OSError:
        hook = None

    mod = types.ModuleType("antenv.axon_hooks")
    mod._hook = hook
    mod.get_axon_ntff_profile_hook = lambda: mod._hook

    def _set(h):
        mod._hook = h

    mod.set_axon_ntff_profile_hook = _set
    sys.modules["antenv.axon_hooks"] = mod
    antenv.axon_hooks = mod


def kernel(x, weight, bias, adjacency):
    from concourse.bass_utils import run_bass_kernel_spmd

    _ensure_axon_profile_hook()
    nc, sched, jorder = _get_graph(adjacency)
    in_maps = _pack_inputs(x, weight, bias, sched, jorder)

    kwargs = {}
    if os.environ.get("KERNEL_TRACE"):
        kwargs["trace"] = True
        tcores = os.environ.get("KERNEL_TRACE_CORES")
        if tcores:
            kwargs["trace_cores"] = [int(t) for t in tcores.split(",")]

    res = run_bass_kernel_spmd(nc, in_maps, core_ids=list(range(N_CORES)),
                               **kwargs)
    kernel.last_result = res
    return _gather_output(res.results)


kernel.last_result = None


# revision 7
# speedup vs baseline: 1.0442x; 1.0442x over previous
"""Trainium2 Bass kernel for AnatomicalMaskedLinear (block-masked dense layer).

Reference op:
    mask  = kron(adjacency, ones(256, 128))            # (21*256, 21*128)
    y     = x.reshape(B, 21*128) @ (weight*mask).T + bias
    out   = y.reshape(B, 21, 256)

Strategy (v2):
  * Only nonzero (256o x 128k) blocks are shipped/matmul'd (S=nnz blocks).
  * 8 cores = 4 batch quarters x 2 node-row halves; all cores share one
    SPMD graph (same adjacency-derived schedule), only data differs.
  * bf16 operands, 1024-wide moving operand: one matmul per (node, j)
    block into a 2-bank PSUM tile [128, 1024] -> 237 matmuls/core instead
    of 474 (less NX issue overhead, fewer LDWEIGHTS).
  * PE warm-up: HAM clock gate holds the PE at 1.2 GHz until ~3.4us of
    sustained activity. A memset + dummy-matmul burst issued before the
    first real data arrives flips the gate early so the real stream runs
    at 2.4 GHz almost immediately.
  * All loads ride the sync HWDGE ring in first-use order (x is packed
    host-side in first-use block order so bulk triggers are contiguous);
    the first node's blocks use per-block triggers so the first matmul
    fires as soon as ~0.5 MB has landed instead of waiting for a bulk
    transfer. Everything stays resident in SBUF (13 MB total).
  * PSUM evacuation (bias add + f32->bf16 cast, activation Identity with
    per-partition bias) AND the store trigger both run on the scalar
    engine: evac->store needs no cross-engine event, and output traffic
    halves (bf16). Host upcasts to f32.
  * Fewer DMA triggers + fewer cross-engine events shrink the end-of-NEFF
    event-semaphore drain ladder, which counts toward measured exec time.
"""

import os
import numpy as np
import ml_dtypes

NUM_NODES = 21
IN_F = 128
OUT_F = 256
BATCH = 4096
N_CORES = 8
P_BATCH = 4                      # batch ways
B_C = BATCH // P_BATCH           # 1024 batch rows per core
K_TOTAL = NUM_NODES * IN_F       # 2688
O_C = NUM_NODES * 128            # 2688 out rows per core (half of each node)
N_WARM = 20                      # dummy matmuls to open the HAM clock gate

_CACHE = {}                      # schedule key -> (nc, sched, jorder)


def _node_order(active):
    """Greedy: minimize newly-required x blocks at each step."""
    loaded = set()
    remaining = set(range(NUM_NODES))
    order = []
    while remaining:
        nxt = min(remaining,
                  key=lambda i: (len(set(active[i]) - loaded), len(active[i]), i))
        order.append(nxt)
        loaded |= set(active[nxt])
        remaining.remove(nxt)
    return order


def _build_schedule(adjacency):
    """[(i, [j...], zero_pad)] in greedy node order; >=1 slot per node.

    Within each node, already-loaded x blocks come first (in load order) and
    newly-required blocks last, so a chain's early matmuls never wait on the
    blocks still streaming in for its tail.
    """
    A = np.asarray(adjacency) != 0
    active = {i: [int(j) for j in np.where(A[i])[0]] for i in range(NUM_NODES)}
    order = _node_order(active)
    jorder = []
    seen = set()
    for i in order:
        for j in active[i]:
            if j not in seen:
                seen.add(j)
                jorder.append(j)
    if not jorder:
        jorder = [0]
    pos = {j: p for p, j in enumerate(jorder)}
    sched = []
    loaded = set()
    for i in order:
        js = active[i]
        if js:
            olds = sorted((j for j in js if j in loaded), key=lambda j: pos[j])
            news = sorted((j for j in js if j not in loaded), key=lambda j: pos[j])
            loaded |= set(js)
            sched.append((i, tuple(olds + news), False))
        else:
            sched.append((i, (jorder[0],), True))
    return tuple(sched), tuple(jorder)


def _w_chunks(sched):
    """Group schedule positions into w DMA chunks: per-node for the first
    few nodes, then ~1MB merged chunks."""
    sizes = [len(js) for _, js, _ in sched]
    chunks = []  # (start_slot, n_slots, last_sched_idx)
    k = 0
    s0 = 0
    while k < len(sched):
        if k < 7:
            n = sizes[k]
            last = k
            k += 1
        else:
            n = 0
            last = k
            while k < len(sched) and (n == 0 or n + sizes[k] <= 34):
                n += sizes[k]
                last = k
                k += 1
        chunks.append((s0, n, last))
        s0 += n
    return chunks


def _build_graph(sched, jorder):
    import concourse.tile as tile
    from concourse import bacc, mybir

    S = sum(len(js) for _, js, _ in sched)
    f32 = mybir.dt.float32
    bf16 = mybir.dt.bfloat16
    pos = {j: p for p, j in enumerate(jorder)}
    NJ = len(jorder)

    nc = bacc.Bacc("TRN2", target_bir_lowering=False, debug=False,
                   num_devices=N_CORES)

    xt_d = nc.declare_dram_parameter("xt", [128, NJ * B_C], bf16, isOutput=False)
    wp_d = nc.declare_dram_parameter("wp", [128, S * 128], bf16, isOutput=False)
    bias_d = nc.declare_dram_parameter("biasr", [128, NUM_NODES], f32,
                                       isOutput=False)
    out_d = nc.declare_dram_parameter("out", [O_C, B_C], bf16, isOutput=True)

    # per-node newly needed x blocks (contiguous in jorder by construction)
    new_js = []
    seen = set()
    for i, js, zero in sched:
        cur = [] if zero else [j for j in js if j not in seen]
        seen |= set(cur)
        new_js.append(cur)

    wchunks = _w_chunks(sched)

    with tile.TileContext(nc) as tc:
        with (
            tc.tile_pool(name="const", bufs=1) as constp,
            tc.tile_pool(name="persist", bufs=1) as persist,
            tc.tile_pool(name="psum", bufs=4, space="PSUM") as psump,
            tc.tile_pool(name="outp", bufs=8) as outp,
        ):
            # --- PE warm-up: memset a small bf16 tile, then dummy matmuls
            # (HAM clock gate: PE runs at 1.2 GHz until ~3.4us of sustained
            # activity; dummies during the DMA wait flip it early) ---
            warm_sb = constp.tile([128, 128], bf16)
            nc.gpsimd.memset(warm_sb[:], 0.0)
            warm_ps = psump.tile([128, B_C], f32, tag="acc", name="warm")
            for t in range(N_WARM):
                nc.tensor.matmul(warm_ps[:, 0:128], warm_sb[:], warm_sb[:],
                                 start=True, stop=True)

            bias_sb = constp.tile([128, NUM_NODES], f32)
            xt_bf = persist.tile([128, NJ * B_C], bf16)
            wp_sb = persist.tile([128, S * 128], bf16)

            # --- load plan: head alternates sync/scalar HWDGE rings so the
            # first chains' data lands ~2x faster; tail rides sync so the
            # scalar engine is free for evac+store from ~11us on ---
            nc.scalar.dma_start(out=bias_sb[:], in_=bias_d[:])

            emitted_w = 0   # how many w chunks issued
            def emit_w_chunk(eng):
                nonlocal emitted_w
                if emitted_w < len(wchunks):
                    s0, n, _last = wchunks[emitted_w]
                    eng.dma_start(out=wp_sb[:, s0 * 128:(s0 + n) * 128],
                                  in_=wp_d[:, s0 * 128:(s0 + n) * 128])
                    emitted_w += 1

            def emit_x(node_idx, eng, per_block=False):
                cur = new_js[node_idx]
                if not cur:
                    return
                p0 = pos[cur[0]]
                n = len(cur)
                if per_block:
                    for t in range(n):
                        e = (nc.sync, nc.scalar)[t % 2]
                        e.dma_start(
                            out=xt_bf[:, (p0 + t) * B_C:(p0 + t + 1) * B_C],
                            in_=xt_d[:, (p0 + t) * B_C:(p0 + t + 1) * B_C])
                else:
                    eng.dma_start(
                        out=xt_bf[:, p0 * B_C:(p0 + n) * B_C],
                        in_=xt_d[:, p0 * B_C:(p0 + n) * B_C])

            # head: node0's w on scalar, its x blocks alternating singles
            emit_w_chunk(nc.scalar)
            emit_x(0, None, per_block=True)
            k = 1
            hx = 0
            while emitted_w < len(wchunks):
                _s0, _n, last = wchunks[emitted_w]
                emit_w_chunk(nc.sync)
                while k <= last:
                    if new_js[k]:
                        hx += 1
                    emit_x(k, nc.scalar if hx <= 2 else nc.sync)
                    k += 1

            # --- compute: per node, two interleaved 512-wide accumulation
            # chains into one 2-bank PSUM tile, single wide evac + store.
            # The last node runs its two half-chains SEQUENTIALLY so the
            # first half's evac+store overlaps the second half's matmuls,
            # leaving only half an evac+store exposed after the last MM. ---
            s0 = 0
            n_sched = len(sched)
            for kk, (i, js, _zero) in enumerate(sched):
                nj = len(js)
                ps = psump.tile([128, B_C], f32, tag="acc", name=f"acc_{i}")
                ot = outp.tile([128, B_C], bf16, tag="ot")
                H = B_C // 2
                if kk >= n_sched - 1:
                    for bt in range(2):
                        for idx, j in enumerate(js):
                            nc.tensor.matmul(
                                ps[:, bt * H:(bt + 1) * H],
                                wp_sb[:, (s0 + idx) * 128:(s0 + idx + 1) * 128],
                                xt_bf[:, pos[j] * B_C + bt * H:
                                      pos[j] * B_C + (bt + 1) * H],
                                start=(idx == 0),
                                stop=(idx == nj - 1),
                            )
                        nc.scalar.add(ot[:, bt * H:(bt + 1) * H],
                                      ps[:, bt * H:(bt + 1) * H],
                                      bias_sb[:, i:i + 1])
                        nc.scalar.dma_start(
                            out=out_d[i * 128:(i + 1) * 128, bt * H:(bt + 1) * H],
                            in_=ot[:, bt * H:(bt + 1) * H])
                else:
                    for idx, j in enumerate(js):
                        w_ap = wp_sb[:, (s0 + idx) * 128:(s0 + idx + 1) * 128]
                        for bt in range(2):
                            nc.tensor.matmul(
                                ps[:, bt * H:(bt + 1) * H],
                                w_ap,
                                xt_bf[:, pos[j] * B_C + bt * H:
                                      pos[j] * B_C + (bt + 1) * H],
                                start=(idx == 0),
                                stop=(idx == nj - 1),
                            )
                    nc.scalar.add(ot[:], ps[:], bias_sb[:, i:i + 1])
                    nc.scalar.dma_start(
                        out=out_d[i * 128:(i + 1) * 128, :], in_=ot[:])
                s0 += nj

    nc.compile()
    return nc


def _get_graph(adjacency):
    sched, jorder = _build_schedule(adjacency)
    key = (sched, jorder)
    if key not in _CACHE:
        _CACHE[key] = (_build_graph(sched, jorder), sched, jorder)
    return _CACHE[key]


def _pack_inputs(x, weight, bias, sched, jorder):
    """Build the 8 per-core input maps (host-side slicing/layout only)."""
    bf16 = ml_dtypes.bfloat16
    x = np.asarray(x, dtype=np.float32)
    weight = np.asarray(weight, dtype=np.float32)
    bias = np.asarray(bias, dtype=np.float32)

    flat = []  # (i, j, zero) in slot order
    for i, js, zero in sched:
        for j in js:
            flat.append((i, j, zero))
    S = len(flat)

    w5 = weight.reshape(NUM_NODES, 2, 128, NUM_NODES, IN_F)  # i, h, o, j, k
    w5t = w5.transpose(1, 4, 0, 3, 2)                        # h, k, i, j, o

    si = np.array([f[0] for f in flat])
    sj = np.array([f[1] for f in flat])
    szero = np.array([f[2] for f in flat])

    wp_h = []
    for h in range(2):
        wp = np.ascontiguousarray(w5t[h][:, si, sj, :])      # [128, S, 128]
        if szero.any():
            wp[:, szero, :] = 0.0
        wp_h.append(wp.reshape(128, S * 128).astype(bf16))

    bias3 = bias.reshape(NUM_NODES, 2, 128)
    bias_h = [np.ascontiguousarray(bias3[:, h, :].T) for h in range(2)]

    jord = list(jorder)
    in_maps = []
    xt_cache = {}
    for c in range(N_CORES):
        bq, h = divmod(c, 2)
        if bq not in xt_cache:
            # [128 k, NJ blocks (first-use order), 1024 batch]
            xq = x[bq * B_C:(bq + 1) * B_C][:, jord, :]      # [1024, NJ, 128]
            xt_cache[bq] = np.ascontiguousarray(
                xq.transpose(2, 1, 0).reshape(128, len(jord) * B_C)).astype(bf16)
        in_maps.append({
            "xt": xt_cache[bq],
            "wp": wp_h[h],
            "biasr": bias_h[h],
        })
    return in_maps


def _gather_output(results):
    y = np.empty((P_BATCH, B_C, NUM_NODES, 2, 128), dtype=np.float32)
    for c in range(N_CORES):
        bq, h = divmod(c, 2)
        oc = results[c]["out"].astype(np.float32).reshape(NUM_NODES, 128, B_C)
        y[bq, :, :, h, :] = oc.transpose(2, 0, 1)
    return y.reshape(BATCH, NUM_NODES, OUT_F)


def _ensure_axon_profile_hook():
    """Provide antenv.axon_hooks if the image lacks it (no-op otherwise).

    concourse.bass_utils imports antenv.axon_hooks on the trace path; some
    images miss the module, which would turn BASS_TRACE=1 into an
    ImportError. Registers the standard ctypes NTFF hook when possible.
    """
    try:
        import antenv.axon_hooks  # noqa: F401
        return
    except ImportError:
        pass
    try:
        import antenv
    except ImportError:
        return
    import contextlib
    import ctypes
    import sys
    import types

    hook = None
    try:
        lib = ctypes.CDLL("/opt/axon/libaxon_pjrt.so")
        if hasattr(lib, "axon_start_nrt_profile"):
            lib.axon_start_nrt_profile.argtypes = [
                ctypes.POINTER(ctypes.c_int64), ctypes.c_size_t]
            lib.axon_start_nrt_profile.restype = ctypes.c_int64
            lib.axon_stop_nrt_profile.argtypes = [ctypes.c_char_p]
            lib.axon_stop_nrt_profile.restype = ctypes.c_int64

            @contextlib.contextmanager
            def hook(output_dir, device_ids):
                import jax
                jax.devices()
                if device_ids:
                    ids = (ctypes.c_int64 * len(device_ids))(*device_ids)
                    rc = lib.axon_start_nrt_profile(ids, len(device_ids))
                else:
                    rc = lib.axon_start_nrt_profile(None, 0)
                if rc != 0:
                    raise RuntimeError(f"axon_start_nrt_profile rc={rc}")
                try:
                    yield
                finally:
                    lib.axon_stop_nrt_profile(str(output_dir).encode())
    except OSError:
        hook = None

    mod = types.ModuleType("antenv.axon_hooks")
    mod._hook = hook
    mod.get_axon_ntff_profile_hook = lambda: mod._hook

    def _set(h):
        mod._hook = h

    mod.set_axon_ntff_profile_hook = _set
    sys.modules["antenv.axon_hooks"] = mod
    antenv.axon_hooks = mod


def kernel(x, weight, bias, adjacency):
    from concourse.bass_utils import run_bass_kernel_spmd

    _ensure_axon_profile_hook()
    nc, sched, jorder = _get_graph(adjacency)
    in_maps = _pack_inputs(x, weight, bias, sched, jorder)

    kwargs = {}
    if os.environ.get("KERNEL_TRACE"):
        kwargs["trace"] = True
        tcores = os.environ.get("KERNEL_TRACE_CORES")
        if tcores:
            kwargs["trace_cores"] = [int(t) for t in tcores.split(",")]

    res = run_bass_kernel_spmd(nc, in_maps, core_ids=list(range(N_CORES)),
                               **kwargs)
    kernel.last_result = res
    return _gather_output(res.results)


kernel.last_result = None
